# revision 20
# baseline (speedup 1.0000x reference)
"""Trainium2 Bass kernel for masked spatial attention softmax.

Computes S = softmax((F_a@Wq.T + bq) @ (F_s@Wk.T + bk).T / sqrt(d) + mask)
over 8 NeuronCores, data-parallel over batch.

Algebra: QK = (F_a @ Wc + bc) @ F_s.T with Wc = Wq.T @ Wk / sqrt(d) and
bc = bq @ Wk / sqrt(d) folded on the host; the bk term is constant along
the softmax axis and drops out of the softmax.  K_s is never materialized.

Host-side input prep (the same make_in_maps step that casts to bf16,
shards over cores, and builds the additive mask) also lays F_a and F_s
out transposed, so the device program runs no PE transposes and no PSUM
evictions at all: PE does QK + the rank-1 additive mask + one small
projection; Scalar does exp over [128, 2048] PSUM chunks with fused
row-sum accumulation; DVE does bias-add + normalize; Sync issues all
DMA (the last batch's stores ride the Scalar queue family instead, to
drain the tail across both HWDGE paths).
"""

import math
from contextlib import ExitStack

import numpy as np
import ml_dtypes

import concourse.bass as bass
import concourse.tile as tile
from concourse import bacc, mybir

# Problem shapes (hardcoded per contract; spec: B=32, T=256, HW=4096, d=256)
B_FULL = 32
N_CORES = 8
BS = B_FULL // N_CORES  # batches per core
T = 256
HW = 4096
D = 256
CK = 2048  # QK chunk width (4 PSUM banks)
NCK = HW // CK
SCALE = 1.0 / math.sqrt(D)  # 1/16
MASK_NEG = -80.0  # exp(-80 + max_logit) << 1e-30; stays in ACT exp valid range

F32 = mybir.dt.float32
BF16 = mybir.dt.bfloat16


def _build_body(tc, ctx, F_aT, F_sT, mbig, Wc, bc, S):
    nc = tc.nc

    singles = ctx.enter_context(tc.tile_pool(name="singles", bufs=1))
    fst_pool = ctx.enter_context(tc.tile_pool(name="fst", bufs=3))
    qpool = ctx.enter_context(tc.tile_pool(name="qpool", bufs=2))
    spool = ctx.enter_context(tc.tile_pool(name="spool", bufs=4))
    opool = ctx.enter_context(tc.tile_pool(name="opool", bufs=2))
    stats = ctx.enter_context(tc.tile_pool(name="stats", bufs=4))
    psum_qk = ctx.enter_context(tc.tile_pool(name="psum_qk", bufs=2, space="PSUM"))

    fat_t, qct_t, fst_t = {}, {}, {}

    # ---- prologue loads: first QK chunk's deps lead the sync queue ----
    # fst is split into lo/hi half-tiles (Tile deps are tile-granular, so
    # chunk 0 must not wait for the hi half's transfer)
    def fst_tiles():
        return (
            fst_pool.tile([128, 2, CK], BF16, tag="fstlo", name="fstlo"),
            fst_pool.tile([128, 2, CK], BF16, tag="fsthi", name="fsthi"),
        )

    fst0 = fst_tiles()
    for ci in range(2):
        nc.sync.dma_start(
            out=fst0[0][:, ci, :], in_=F_sT[0, ci * 128:(ci + 1) * 128, 0:CK]
        )
    fst_t[0] = fst0

    fat0 = qpool.tile([128, 2, T], BF16, tag="fat", name="fat")
    nc.sync.dma_start(
        out=fat0[:], in_=F_aT[0].rearrange("(dh dl) t -> dl dh t", dl=128)
    )
    fat_t[0] = fat0

    wc_sb = singles.tile([128, 2, D], BF16, tag="wc", name="wc")
    nc.sync.dma_start(out=wc_sb[:], in_=Wc.rearrange("(kh kl) o -> kl kh o", kl=128))

    for ci in range(2):
        nc.sync.dma_start(
            out=fst0[1][:, ci, :], in_=F_sT[0, ci * 128:(ci + 1) * 128, CK:HW]
        )

    # Scalar queue: bias + mask rows (first needed at the first exp/QK-mask)
    bc_sb = singles.tile([128, 2], F32, tag="bc", name="bc")
    nc.scalar.dma_start(out=bc_sb[:], in_=bc.rearrange("(a p) -> p a", p=128))
    mb_sb = singles.tile([1, BS * HW], BF16, tag="mb", name="mb")
    nc.scalar.dma_start(out=mb_sb[:], in_=mbig.rearrange("b s -> (b s)")[None, :])

    ones16 = singles.tile([1, 128], BF16, tag="ones16", name="ones16")
    nc.vector.memset(ones16[:], 1.0)

    def load_batch(b):
        """Prefetch F_a[b].T (small, first) and F_s[b].T per ci, lo then hi."""
        fat = qpool.tile([128, 2, T], BF16, tag="fat", name="fat")
        nc.sync.dma_start(
            out=fat[:], in_=F_aT[b].rearrange("(dh dl) t -> dl dh t", dl=128)
        )
        fat_t[b] = fat
        fst = fst_tiles()
        for h in range(2):
            for ci in range(2):
                nc.sync.dma_start(
                    out=fst[h][:, ci, :],
                    in_=F_sT[b, ci * 128:(ci + 1) * 128, h * CK:(h + 1) * CK],
                )
        fst_t[b] = fst

    def qchain(b):
        """Q~.T = Wc.T @ F_a.T + bc (scale prefolded), bf16.  One PSUM tile
        (two different banks) for both halves: a single pool rotation."""
        fat = fat_t.pop(b)
        qct = qpool.tile([128, 2, T], BF16, tag="qct", name="qct")
        pj = psum_qk.tile([128, CK], F32, tag="pq", name="pq")
        for m in range(2):  # d_out tile
            sl = slice(m * 512, m * 512 + T)
            for k in range(2):  # d_in tile
                nc.tensor.matmul(
                    pj[:, sl],
                    wc_sb[:, k, m * 128:(m + 1) * 128],
                    fat[:, k, :],
                    start=(k == 0),
                    stop=(k == 1),
                )
        for m in range(2):
            nc.vector.tensor_scalar_add(
                out=qct[:, m, :], in0=pj[:, m * 512:m * 512 + T],
                scalar1=bc_sb[:, m:m + 1],
            )
        qct_t[b] = qct

    def qk_chunk(b, tt, ck, s_prs, st, fine=False):
        """QK + mask for one [128, 2048] chunk (4 PSUM banks), exp→bf16 with
        fused masked-rowsum accumulation.  fine=True splits exp in two
        [128, 1024] halves (finer epilogue pipelining for the last rowtile)."""
        fst = fst_t[b][ck]
        qct = qct_t[b]
        pq = psum_qk.tile([128, CK], F32, tag="pq", name="pq")
        # weight-reuse ordering: all four 512-banks grouped by lhsT (qct ci)
        for ci in range(2):
            for h in range(4):  # 512-wide quarter = one PSUM bank
                nc.tensor.matmul(
                    pq[:, h * 512:(h + 1) * 512],
                    qct[:, ci, tt * 128:(tt + 1) * 128],
                    fst[:, ci, h * 512:(h + 1) * 512],
                    start=(ci == 0),
                    stop=False,
                )
        for h in range(4):
            mb0 = b * HW + ck * CK + h * 512
            nc.tensor.matmul(
                pq[:, h * 512:(h + 1) * 512],
                ones16[:],
                mb_sb[:, mb0:mb0 + 512],
                start=False,
                stop=True,
            )
        s_pr = spool.tile([128, CK], BF16, tag="s", name="s")
        if fine:
            for q in range(2):
                nc.scalar.activation(
                    out=s_pr[:, q * 1024:(q + 1) * 1024],
                    in_=pq[:, q * 1024:(q + 1) * 1024],
                    func=mybir.ActivationFunctionType.Exp,
                    accum_out=st[:, 2 * ck + q:2 * ck + q + 1],
                )
        else:
            nc.scalar.activation(
                out=s_pr[:],
                in_=pq[:],
                func=mybir.ActivationFunctionType.Exp,
                accum_out=st[:, ck:ck + 1],
            )
        s_prs.append(s_pr)

    def finish_rowtile(b, tt, s_prs, st):
        rowsum = stats.tile([128, 1], F32, tag="rowsum", name="rowsum")
        nc.vector.reduce_sum(
            out=rowsum[:], in_=st[:, 0:NCK], axis=mybir.AxisListType.X
        )
        recip = stats.tile([128, 1], F32, tag="recip", name="recip")
        nc.vector.reciprocal(out=recip[:], in_=rowsum[:])
        o_tile = opool.tile([128, HW], BF16, tag="o", name="o")
        for h in range(NCK):
            sl = slice(h * CK, (h + 1) * CK)
            nc.vector.tensor_scalar_mul(
                out=o_tile[:, sl], in0=s_prs[h][:], scalar1=recip[:, 0:1]
            )
            nc.sync.dma_start(
                out=S[b, tt * 128:(tt + 1) * 128, sl], in_=o_tile[:, sl]
            )

    def finish_rowtile_fine(b, tt, s_prs, st):
        """Quarter-granular epilogue for the very last rowtile: normalize and
        store [128, 1024] pieces, alternating HWDGE queues, so the tail is
        one quarter deep instead of one rowtile deep."""
        rowsum = stats.tile([128, 1], F32, tag="rowsum", name="rowsum")
        nc.vector.reduce_sum(out=rowsum[:], in_=st[:], axis=mybir.AxisListType.X)
        recip = stats.tile([128, 1], F32, tag="recip", name="recip")
        nc.vector.reciprocal(out=recip[:], in_=rowsum[:])
        o_tile = opool.tile([128, HW], BF16, tag="o", name="o")
        for q in range(4):
            sl = slice(q * 1024, (q + 1) * 1024)
            nc.vector.tensor_scalar_mul(
                out=o_tile[:, sl],
                in0=s_prs[q // 2][:, (q % 2) * 1024:(q % 2 + 1) * 1024],
                scalar1=recip[:, 0:1],
            )
            eng = nc.scalar if q % 2 == 0 else nc.sync
            eng.dma_start(
                out=S[b, tt * 128:(tt + 1) * 128, sl], in_=o_tile[:, sl]
            )

    # ---- software pipeline ----
    qchain(0)
    load_batch(1)

    for b in range(BS):
        for tt in range(2):
            fine = b == BS - 1 and tt == 1
            s_prs = []
            st = stats.tile([128, 2 * NCK], F32, tag="st", name="st")
            for ck in range(NCK):
                qk_chunk(b, tt, ck, s_prs, st, fine=fine)
                # stage prefetch + next Q-chain into fixed slots
                if tt == 0 and ck == 1 and b + 2 < BS:
                    load_batch(b + 2)
                elif tt == 1 and ck == 0 and b + 1 < BS:
                    qchain(b + 1)
            if fine:
                finish_rowtile_fine(b, tt, s_prs, st)
            else:
                finish_rowtile(b, tt, s_prs, st)
        fst_t.pop(b, None)
        qct_t.pop(b, None)


def build_nc():
    nc = bacc.Bacc(
        "TRN2",
        target_bir_lowering=False,
        debug=False,
        num_devices=N_CORES,
    )
    F_aT = nc.dram_tensor("F_aT", [BS, D, T], BF16, kind="ExternalInput")
    F_sT = nc.dram_tensor("F_sT", [BS, D, HW], BF16, kind="ExternalInput")
    mbig = nc.dram_tensor("mbig", [BS, HW], BF16, kind="ExternalInput")
    Wc = nc.dram_tensor("Wc", [D, D], BF16, kind="ExternalInput")
    bc = nc.dram_tensor("bc", [D], F32, kind="ExternalInput")
    S = nc.dram_tensor("S", [BS, T, HW], BF16, kind="ExternalOutput")

    with tile.TileContext(nc) as tc, ExitStack() as ctx:
        _build_body(
            tc, ctx, F_aT.ap(), F_sT.ap(), mbig.ap(), Wc.ap(), bc.ap(), S.ap()
        )
    nc.compile()
    return nc


def make_in_maps(F_a, F_s, M_s, Wq, bq, Wk):
    F_a = np.asarray(F_a, dtype=np.float32).astype(ml_dtypes.bfloat16)
    F_s = np.asarray(F_s, dtype=np.float32).astype(ml_dtypes.bfloat16)
    M_s = np.asarray(M_s)
    Wqf = np.asarray(Wq, dtype=np.float32)
    Wkf = np.asarray(Wk, dtype=np.float32)
    bqf = np.asarray(bq, dtype=np.float32)
    # Fold: Q~ = F_a @ Wc + bc with scale pre-applied (host-side weights math)
    Wc = np.ascontiguousarray(
        ((Wqf.T @ Wkf) * np.float32(SCALE)).astype(ml_dtypes.bfloat16)
    )
    bc = np.ascontiguousarray(((bqf @ Wkf) * np.float32(SCALE)).astype(np.float32))

    # device-friendly transposed layouts (d on the partition axis)
    F_aT = np.ascontiguousarray(F_a.transpose(0, 2, 1))  # [B, d, T]
    F_sT = np.ascontiguousarray(F_s.transpose(0, 2, 1))  # [B, d, HW]

    m = M_s.reshape(M_s.shape[0], -1) == 1  # [B, HW]
    mbig = np.where(m, np.float32(0.0), np.float32(MASK_NEG)).astype(
        ml_dtypes.bfloat16
    )

    in_maps = []
    for i in range(N_CORES):
        sl = slice(i * BS, (i + 1) * BS)
        in_maps.append(
            dict(
                F_aT=np.ascontiguousarray(F_aT[sl]),
                F_sT=np.ascontiguousarray(F_sT[sl]),
                mbig=np.ascontiguousarray(mbig[sl]),
                Wc=Wc,
                bc=bc,
            )
        )
    return in_maps


_NC_CACHE = None


def _get_nc():
    global _NC_CACHE
    if _NC_CACHE is None:
        _NC_CACHE = build_nc()
    return _NC_CACHE


def run(in_maps, **kwargs):
    from concourse import bass_utils

    nc = _get_nc()
    res = bass_utils.run_bass_kernel_spmd(
        nc, in_maps, core_ids=list(range(N_CORES)), **kwargs
    )
    return res


def kernel(F_a, F_s, M_s, Wq, bq, Wk, bk):
    in_maps = make_in_maps(F_a, F_s, M_s, Wq, bq, Wk)
    res = run(in_maps)
    return np.concatenate(
        [np.asarray(r["S"]).astype(np.float32) for r in res.results], axis=0
    )


# revision 27
# speedup vs baseline: 1.0368x; 1.0368x over previous
"""Trainium2 Bass kernel for masked spatial attention softmax.

Computes S = softmax((F_a@Wq.T + bq) @ (F_s@Wk.T + bk).T / sqrt(d) + mask)
over 8 NeuronCores, data-parallel over batch.

Algebra: QK = (F_a @ Wc + bc) @ F_s.T with Wc = Wq.T @ Wk / sqrt(d) and
bc = bq @ Wk / sqrt(d) folded on the host; the bk term is constant along
the softmax axis and drops out of the softmax.  K_s is never materialized.

Host-side input prep (the same make_in_maps step that casts to bf16,
shards over cores, and builds the additive mask) also lays F_a and F_s
out transposed, so the device program runs no PE transposes and no PSUM
evictions at all: PE does QK + the rank-1 additive mask + one small
projection; Scalar does exp over [128, 2048] PSUM chunks with fused
row-sum accumulation; DVE does bias-add + normalize; Sync issues all
DMA (the last batch's stores ride the Scalar queue family instead, to
drain the tail across both HWDGE paths).
"""

import math
from contextlib import ExitStack

import numpy as np
import ml_dtypes

import concourse.bass as bass
import concourse.tile as tile
from concourse import bacc, mybir

# Problem shapes (hardcoded per contract; spec: B=32, T=256, HW=4096, d=256)
B_FULL = 32
N_CORES = 8
BS = B_FULL // N_CORES  # batches per core
T = 256
HW = 4096
D = 256
CK = 2048  # QK chunk width (4 PSUM banks)
NCK = HW // CK
SCALE = 1.0 / math.sqrt(D)  # 1/16
MASK_NEG = -80.0  # exp(-80 + max_logit) << 1e-30; stays in ACT exp valid range

F32 = mybir.dt.float32
BF16 = mybir.dt.bfloat16


def _build_body(tc, ctx, F_aT, F_sT, mbig, Wc, bc, S):
    nc = tc.nc

    singles = ctx.enter_context(tc.tile_pool(name="singles", bufs=1))
    fst_pool = ctx.enter_context(tc.tile_pool(name="fst", bufs=3))
    qpool = ctx.enter_context(tc.tile_pool(name="qpool", bufs=2))
    spool = ctx.enter_context(tc.tile_pool(name="spool", bufs=4))
    opool = ctx.enter_context(tc.tile_pool(name="opool", bufs=2))
    stats = ctx.enter_context(tc.tile_pool(name="stats", bufs=4))
    psum_qk = ctx.enter_context(tc.tile_pool(name="psum_qk", bufs=2, space="PSUM"))

    fat_t, qct_t, fst_t = {}, {}, {}

    # ---- prologue loads: first QK chunk's deps lead the sync queue ----
    # fst is split into lo/hi half-tiles (Tile deps are tile-granular, so
    # chunk 0 must not wait for the hi half's transfer)
    def fst_tiles():
        return (
            fst_pool.tile([128, 2, CK], BF16, tag="fstlo", name="fstlo"),
            fst_pool.tile([128, 2, CK], BF16, tag="fsthi", name="fsthi"),
        )

    fst0 = fst_tiles()
    for ci in range(2):
        nc.sync.dma_start(
            out=fst0[0][:, ci, :], in_=F_sT[0, ci * 128:(ci + 1) * 128, 0:CK]
        )
    fst_t[0] = fst0

    fat0 = qpool.tile([128, 2, T], BF16, tag="fat", name="fat")
    nc.sync.dma_start(
        out=fat0[:], in_=F_aT[0].rearrange("(dh dl) t -> dl dh t", dl=128)
    )
    fat_t[0] = fat0

    wc_sb = singles.tile([128, 2, D], BF16, tag="wc", name="wc")
    nc.sync.dma_start(out=wc_sb[:], in_=Wc.rearrange("(kh kl) o -> kl kh o", kl=128))

    for ci in range(2):
        nc.sync.dma_start(
            out=fst0[1][:, ci, :], in_=F_sT[0, ci * 128:(ci + 1) * 128, CK:HW]
        )

    # Scalar queue: bias + mask rows (first needed at the first exp/QK-mask)
    bc_sb = singles.tile([128, 2], F32, tag="bc", name="bc")
    nc.scalar.dma_start(out=bc_sb[:], in_=bc.rearrange("(a p) -> p a", p=128))
    mb_sb = singles.tile([1, BS * HW], BF16, tag="mb", name="mb")
    nc.scalar.dma_start(out=mb_sb[:], in_=mbig.rearrange("b s -> (b s)")[None, :])

    ones16 = singles.tile([1, 128], BF16, tag="ones16", name="ones16")
    nc.vector.memset(ones16[:], 1.0)

    def load_batch(b):
        """Prefetch F_a[b].T (small, first) and F_s[b].T per ci, lo then hi."""
        fat = qpool.tile([128, 2, T], BF16, tag="fat", name="fat")
        nc.sync.dma_start(
            out=fat[:], in_=F_aT[b].rearrange("(dh dl) t -> dl dh t", dl=128)
        )
        fat_t[b] = fat
        fst = fst_tiles()
        for h in range(2):
            for ci in range(2):
                nc.sync.dma_start(
                    out=fst[h][:, ci, :],
                    in_=F_sT[b, ci * 128:(ci + 1) * 128, h * CK:(h + 1) * CK],
                )
        fst_t[b] = fst

    def qchain(b):
        """Q~.T = Wc.T @ F_a.T + bc (scale prefolded), bf16.  One PSUM tile
        (two different banks) for both halves: a single pool rotation."""
        fat = fat_t.pop(b)
        qct = qpool.tile([128, 2, T], BF16, tag="qct", name="qct")
        pj = psum_qk.tile([128, CK], F32, tag="pq", name="pq")
        for m in range(2):  # d_out tile
            sl = slice(m * 512, m * 512 + T)
            for k in range(2):  # d_in tile
                nc.tensor.matmul(
                    pj[:, sl],
                    wc_sb[:, k, m * 128:(m + 1) * 128],
                    fat[:, k, :],
                    start=(k == 0),
                    stop=(k == 1),
                )
        for m in range(2):
            nc.vector.tensor_scalar_add(
                out=qct[:, m, :], in0=pj[:, m * 512:m * 512 + T],
                scalar1=bc_sb[:, m:m + 1],
            )
        qct_t[b] = qct

    def qk_chunk(b, tt, ck, s_prs, st, fine=False):
        """QK + mask for one [128, 2048] chunk (4 PSUM banks), exp→bf16 with
        fused masked-rowsum accumulation.  fine=True splits exp in two
        [128, 1024] halves (finer epilogue pipelining for the last rowtile)."""
        fst = fst_t[b][ck]
        qct = qct_t[b]
        pq = psum_qk.tile([128, CK], F32, tag="pq", name="pq")
        # weight-reuse ordering: all four 512-banks grouped by lhsT (qct ci)
        for ci in range(2):
            for h in range(4):  # 512-wide quarter = one PSUM bank
                nc.tensor.matmul(
                    pq[:, h * 512:(h + 1) * 512],
                    qct[:, ci, tt * 128:(tt + 1) * 128],
                    fst[:, ci, h * 512:(h + 1) * 512],
                    start=(ci == 0),
                    stop=False,
                )
        for h in range(4):
            mb0 = b * HW + ck * CK + h * 512
            nc.tensor.matmul(
                pq[:, h * 512:(h + 1) * 512],
                ones16[:],
                mb_sb[:, mb0:mb0 + 512],
                start=False,
                stop=True,
            )
        s_pr = spool.tile([128, CK], BF16, tag="s", name="s")
        if fine:
            for q in range(2):
                nc.scalar.activation(
                    out=s_pr[:, q * 1024:(q + 1) * 1024],
                    in_=pq[:, q * 1024:(q + 1) * 1024],
                    func=mybir.ActivationFunctionType.Exp,
                    accum_out=st[:, 2 * ck + q:2 * ck + q + 1],
                )
        else:
            nc.scalar.activation(
                out=s_pr[:],
                in_=pq[:],
                func=mybir.ActivationFunctionType.Exp,
                accum_out=st[:, ck:ck + 1],
            )
        s_prs.append(s_pr)

    def finish_rowtile(b, tt, s_prs, st):
        rowsum = stats.tile([128, 1], F32, tag="rowsum", name="rowsum")
        nc.vector.reduce_sum(
            out=rowsum[:], in_=st[:, 0:NCK], axis=mybir.AxisListType.X
        )
        recip = stats.tile([128, 1], F32, tag="recip", name="recip")
        nc.vector.reciprocal(out=recip[:], in_=rowsum[:])
        o_tile = opool.tile([128, HW], BF16, tag="o", name="o")
        for h in range(NCK):
            sl = slice(h * CK, (h + 1) * CK)
            nc.vector.tensor_scalar_mul(
                out=o_tile[:, sl], in0=s_prs[h][:], scalar1=recip[:, 0:1]
            )
            nc.sync.dma_start(
                out=S[b, tt * 128:(tt + 1) * 128, sl], in_=o_tile[:, sl]
            )

    def finish_rowtile_fine(b, tt, s_prs, st):
        """Quarter-granular epilogue for the very last rowtile: normalize and
        store [128, 1024] pieces, alternating HWDGE queues, so the tail is
        one quarter deep instead of one rowtile deep."""
        rowsum = stats.tile([128, 1], F32, tag="rowsum", name="rowsum")
        nc.vector.reduce_sum(out=rowsum[:], in_=st[:], axis=mybir.AxisListType.X)
        recip = stats.tile([128, 1], F32, tag="recip", name="recip")
        nc.vector.reciprocal(out=recip[:], in_=rowsum[:])
        o_tile = opool.tile([128, HW], BF16, tag="o", name="o")
        for q in range(4):
            sl = slice(q * 1024, (q + 1) * 1024)
            nc.vector.tensor_scalar_mul(
                out=o_tile[:, sl],
                in0=s_prs[q // 2][:, (q % 2) * 1024:(q % 2 + 1) * 1024],
                scalar1=recip[:, 0:1],
            )
            eng = nc.scalar if q % 2 == 0 else nc.sync
            eng.dma_start(
                out=S[b, tt * 128:(tt + 1) * 128, sl], in_=o_tile[:, sl]
            )

    # ---- software pipeline ----
    qchain(0)
    load_batch(1)

    for b in range(BS):
        for tt in range(2):
            fine = b == BS - 1 and tt == 1
            s_prs = []
            st = stats.tile([128, 2 * NCK], F32, tag="st", name="st")
            for ck in range(NCK):
                qk_chunk(b, tt, ck, s_prs, st, fine=fine)
                # stage prefetch + next Q-chain into fixed slots
                if tt == 0 and ck == 1 and b + 2 < BS:
                    load_batch(b + 2)
                elif tt == 1 and ck == 0 and b + 1 < BS:
                    qchain(b + 1)
            if fine:
                finish_rowtile_fine(b, tt, s_prs, st)
            else:
                finish_rowtile(b, tt, s_prs, st)
        fst_t.pop(b, None)
        qct_t.pop(b, None)


def build_nc():
    nc = bacc.Bacc(
        "TRN2",
        target_bir_lowering=False,
        debug=False,
        num_devices=N_CORES,
    )
    F_aT = nc.dram_tensor("F_aT", [BS, D, T], BF16, kind="ExternalInput")
    F_sT = nc.dram_tensor("F_sT", [BS, D, HW], BF16, kind="ExternalInput")
    mbig = nc.dram_tensor("mbig", [BS, HW], BF16, kind="ExternalInput")
    Wc = nc.dram_tensor("Wc", [D, D], BF16, kind="ExternalInput")
    bc = nc.dram_tensor("bc", [D], F32, kind="ExternalInput")
    S = nc.dram_tensor("S", [BS, T, HW], BF16, kind="ExternalOutput")

    with tile.TileContext(nc) as tc, ExitStack() as ctx:
        _build_body(
            tc, ctx, F_aT.ap(), F_sT.ap(), mbig.ap(), Wc.ap(), bc.ap(), S.ap()
        )
    nc.compile()
    return nc


def make_in_maps(F_a, F_s, M_s, Wq, bq, Wk):
    F_a = np.asarray(F_a, dtype=np.float32).astype(ml_dtypes.bfloat16)
    F_s = np.asarray(F_s, dtype=np.float32).astype(ml_dtypes.bfloat16)
    M_s = np.asarray(M_s)
    Wqf = np.asarray(Wq, dtype=np.float32)
    Wkf = np.asarray(Wk, dtype=np.float32)
    bqf = np.asarray(bq, dtype=np.float32)
    # Fold: Q~ = F_a @ Wc + bc with scale pre-applied (host-side weights math)
    Wc = np.ascontiguousarray(
        ((Wqf.T @ Wkf) * np.float32(SCALE)).astype(ml_dtypes.bfloat16)
    )
    bc = np.ascontiguousarray(((bqf @ Wkf) * np.float32(SCALE)).astype(np.float32))

    # device-friendly transposed layouts (d on the partition axis)
    F_aT = np.ascontiguousarray(F_a.transpose(0, 2, 1))  # [B, d, T]
    F_sT = np.ascontiguousarray(F_s.transpose(0, 2, 1))  # [B, d, HW]

    m = M_s.reshape(M_s.shape[0], -1) == 1  # [B, HW]
    mbig = np.where(m, np.float32(0.0), np.float32(MASK_NEG)).astype(
        ml_dtypes.bfloat16
    )

    in_maps = []
    for i in range(N_CORES):
        sl = slice(i * BS, (i + 1) * BS)
        in_maps.append(
            dict(
                F_aT=np.ascontiguousarray(F_aT[sl]),
                F_sT=np.ascontiguousarray(F_sT[sl]),
                mbig=np.ascontiguousarray(mbig[sl]),
                Wc=Wc,
                bc=bc,
            )
        )
    return in_maps


_NC_CACHE = None


def _get_nc():
    global _NC_CACHE
    if _NC_CACHE is None:
        _NC_CACHE = build_nc()
    return _NC_CACHE


def run(in_maps, **kwargs):
    from concourse import bass_utils

    nc = _get_nc()
    res = bass_utils.run_bass_kernel_spmd(
        nc, in_maps, core_ids=list(range(N_CORES)), **kwargs
    )
    return res


def kernel(F_a, F_s, M_s, Wq, bq, Wk, bk):
    in_maps = make_in_maps(F_a, F_s, M_s, Wq, bq, Wk)
    res = run(in_maps)
    return np.concatenate(
        [np.asarray(r["S"]).astype(np.float32) for r in res.results], axis=0
    )


# revision 28
# speedup vs baseline: 1.0501x; 1.0128x over previous
"""Trainium2 Bass kernel for masked spatial attention softmax.

Computes S = softmax((F_a@Wq.T + bq) @ (F_s@Wk.T + bk).T / sqrt(d) + mask)
over 8 NeuronCores, data-parallel over batch.

Algebra: QK = (F_a @ Wc + bc) @ F_s.T with Wc = Wq.T @ Wk / sqrt(d) and
bc = bq @ Wk / sqrt(d) folded on the host; the bk term is constant along
the softmax axis and drops out of the softmax.  K_s is never materialized.

Host-side input prep (the same make_in_maps step that casts to bf16,
shards over cores, and builds the additive mask) also lays F_a and F_s
out transposed, so the device program runs no PE transposes and no PSUM
evictions at all: PE does QK + the rank-1 additive mask + one small
projection; Scalar does exp over [128, 2048] PSUM chunks with fused
row-sum accumulation; DVE does bias-add + normalize; Sync issues all
DMA (the last batch's stores ride the Scalar queue family instead, to
drain the tail across both HWDGE paths).
"""

import math
from contextlib import ExitStack

import numpy as np
import ml_dtypes

import concourse.bass as bass
import concourse.tile as tile
from concourse import bacc, mybir

# Problem shapes (hardcoded per contract; spec: B=32, T=256, HW=4096, d=256)
B_FULL = 32
N_CORES = 8
BS = B_FULL // N_CORES  # batches per core
T = 256
HW = 4096
D = 256
CK = 2048  # QK chunk width (4 PSUM banks)
NCK = HW // CK
SCALE = 1.0 / math.sqrt(D)  # 1/16
MASK_NEG = -80.0  # exp(-80 + max_logit) << 1e-30; stays in ACT exp valid range

F32 = mybir.dt.float32
BF16 = mybir.dt.bfloat16


def _build_body(tc, ctx, F_aT, F_sT, mbig, Wc, bc, S):
    nc = tc.nc

    singles = ctx.enter_context(tc.tile_pool(name="singles", bufs=1))
    fst_pool = ctx.enter_context(tc.tile_pool(name="fst", bufs=3))
    qpool = ctx.enter_context(tc.tile_pool(name="qpool", bufs=2))
    spool = ctx.enter_context(tc.tile_pool(name="spool", bufs=4))
    opool = ctx.enter_context(tc.tile_pool(name="opool", bufs=2))
    stats = ctx.enter_context(tc.tile_pool(name="stats", bufs=4))
    psum_qk = ctx.enter_context(tc.tile_pool(name="psum_qk", bufs=2, space="PSUM"))

    fat_t, qct_t, fst_t = {}, {}, {}

    # ---- prologue loads: first QK chunk's deps lead the sync queue ----
    # fst is split into lo/hi half-tiles (Tile deps are tile-granular, so
    # chunk 0 must not wait for the hi half's transfer)
    def fst_tiles():
        return (
            fst_pool.tile([128, 2, CK], BF16, tag="fstlo", name="fstlo"),
            fst_pool.tile([128, 2, CK], BF16, tag="fsthi", name="fsthi"),
        )

    fst0 = fst_tiles()
    for ci in range(2):
        nc.sync.dma_start(
            out=fst0[0][:, ci, :], in_=F_sT[0, ci * 128:(ci + 1) * 128, 0:CK]
        )
    fst_t[0] = fst0

    fat0 = qpool.tile([128, 2, T], BF16, tag="fat", name="fat")
    nc.sync.dma_start(
        out=fat0[:], in_=F_aT[0].rearrange("(dh dl) t -> dl dh t", dl=128)
    )
    fat_t[0] = fat0

    wc_sb = singles.tile([128, 2, D], BF16, tag="wc", name="wc")
    nc.sync.dma_start(out=wc_sb[:], in_=Wc.rearrange("(kh kl) o -> kl kh o", kl=128))

    for ci in range(2):
        nc.sync.dma_start(
            out=fst0[1][:, ci, :], in_=F_sT[0, ci * 128:(ci + 1) * 128, CK:HW]
        )

    # Scalar queue: bias + mask rows (first needed at the first exp/QK-mask)
    bc_sb = singles.tile([128, 2], F32, tag="bc", name="bc")
    nc.scalar.dma_start(out=bc_sb[:], in_=bc.rearrange("(a p) -> p a", p=128))
    mb_sb = singles.tile([1, BS * HW], BF16, tag="mb", name="mb")
    nc.scalar.dma_start(out=mb_sb[:], in_=mbig.rearrange("b s -> (b s)")[None, :])

    ones16 = singles.tile([1, 128], BF16, tag="ones16", name="ones16")
    nc.vector.memset(ones16[:], 1.0)

    def load_batch(b):
        """Prefetch F_a[b].T (small, first) and F_s[b].T per ci, lo then hi."""
        fat = qpool.tile([128, 2, T], BF16, tag="fat", name="fat")
        nc.sync.dma_start(
            out=fat[:], in_=F_aT[b].rearrange("(dh dl) t -> dl dh t", dl=128)
        )
        fat_t[b] = fat
        fst = fst_tiles()
        for h in range(2):
            for ci in range(2):
                nc.sync.dma_start(
                    out=fst[h][:, ci, :],
                    in_=F_sT[b, ci * 128:(ci + 1) * 128, h * CK:(h + 1) * CK],
                )
        fst_t[b] = fst

    def qchain(b):
        """Q~.T = Wc.T @ F_a.T + bc (scale prefolded), bf16.  One PSUM tile
        (two different banks) for both halves: a single pool rotation."""
        fat = fat_t.pop(b)
        qct = qpool.tile([128, 2, T], BF16, tag="qct", name="qct")
        pj = psum_qk.tile([128, CK], F32, tag="pq", name="pq")
        for m in range(2):  # d_out tile
            sl = slice(m * 512, m * 512 + T)
            for k in range(2):  # d_in tile
                nc.tensor.matmul(
                    pj[:, sl],
                    wc_sb[:, k, m * 128:(m + 1) * 128],
                    fat[:, k, :],
                    start=(k == 0),
                    stop=(k == 1),
                )
        for m in range(2):
            nc.vector.tensor_scalar_add(
                out=qct[:, m, :], in0=pj[:, m * 512:m * 512 + T],
                scalar1=bc_sb[:, m:m + 1],
            )
        qct_t[b] = qct

    def qk_chunk(b, tt, ck, s_prs, st, fine=False):
        """QK + mask for one [128, 2048] chunk (4 PSUM banks), exp→bf16 with
        fused masked-rowsum accumulation.  fine=True splits exp in two
        [128, 1024] halves (finer epilogue pipelining for the last rowtile)."""
        fst = fst_t[b][ck]
        qct = qct_t[b]
        pq = psum_qk.tile([128, CK], F32, tag="pq", name="pq")
        # weight-reuse ordering: all four 512-banks grouped by lhsT (qct ci)
        for ci in range(2):
            for h in range(4):  # 512-wide quarter = one PSUM bank
                nc.tensor.matmul(
                    pq[:, h * 512:(h + 1) * 512],
                    qct[:, ci, tt * 128:(tt + 1) * 128],
                    fst[:, ci, h * 512:(h + 1) * 512],
                    start=(ci == 0),
                    stop=False,
                )
        for h in range(4):
            mb0 = b * HW + ck * CK + h * 512
            nc.tensor.matmul(
                pq[:, h * 512:(h + 1) * 512],
                ones16[:],
                mb_sb[:, mb0:mb0 + 512],
                start=False,
                stop=True,
            )
        s_pr = spool.tile([128, CK], BF16, tag="s", name="s")
        if fine:
            for q in range(2):
                nc.scalar.activation(
                    out=s_pr[:, q * 1024:(q + 1) * 1024],
                    in_=pq[:, q * 1024:(q + 1) * 1024],
                    func=mybir.ActivationFunctionType.Exp,
                    accum_out=st[:, 2 * ck + q:2 * ck + q + 1],
                )
        else:
            nc.scalar.activation(
                out=s_pr[:],
                in_=pq[:],
                func=mybir.ActivationFunctionType.Exp,
                accum_out=st[:, ck:ck + 1],
            )
        s_prs.append(s_pr)

    def finish_rowtile(b, tt, s_prs, st):
        rowsum = stats.tile([128, 1], F32, tag="rowsum", name="rowsum")
        nc.vector.reduce_sum(
            out=rowsum[:], in_=st[:, 0:NCK], axis=mybir.AxisListType.X
        )
        recip = stats.tile([128, 1], F32, tag="recip", name="recip")
        nc.vector.reciprocal(out=recip[:], in_=rowsum[:])
        o_tile = opool.tile([128, HW], BF16, tag="o", name="o")
        for h in range(NCK):
            sl = slice(h * CK, (h + 1) * CK)
            nc.vector.tensor_scalar_mul(
                out=o_tile[:, sl], in0=s_prs[h][:], scalar1=recip[:, 0:1]
            )
            nc.sync.dma_start(
                out=S[b, tt * 128:(tt + 1) * 128, sl], in_=o_tile[:, sl]
            )

    def finish_rowtile_fine(b, tt, s_prs, st):
        """Quarter-granular epilogue for the very last rowtile: normalize and
        store [128, 1024] pieces, alternating HWDGE queues, so the tail is
        one quarter deep instead of one rowtile deep."""
        rowsum = stats.tile([128, 1], F32, tag="rowsum", name="rowsum")
        nc.vector.reduce_sum(out=rowsum[:], in_=st[:], axis=mybir.AxisListType.X)
        recip = stats.tile([128, 1], F32, tag="recip", name="recip")
        nc.vector.reciprocal(out=recip[:], in_=rowsum[:])
        o_tile = opool.tile([128, HW], BF16, tag="o", name="o")
        for q in range(4):
            sl = slice(q * 1024, (q + 1) * 1024)
            nc.vector.tensor_scalar_mul(
                out=o_tile[:, sl],
                in0=s_prs[q // 2][:, (q % 2) * 1024:(q % 2 + 1) * 1024],
                scalar1=recip[:, 0:1],
            )
            eng = nc.scalar if q % 2 == 0 else nc.sync
            eng.dma_start(
                out=S[b, tt * 128:(tt + 1) * 128, sl], in_=o_tile[:, sl]
            )

    # ---- software pipeline ----
    qchain(0)
    load_batch(1)

    for b in range(BS):
        for tt in range(2):
            fine = b == BS - 1 and tt == 1
            s_prs = []
            st = stats.tile([128, 2 * NCK], F32, tag="st", name="st")
            for ck in range(NCK):
                # next Q-chain a half-rowtile earlier for b>=1: qct(b+1) must
                # beat batch b+1's first chunk or PE stalls ~1.7us per batch
                if tt == 0 and ck == 0 and 1 <= b and b + 1 < BS:
                    qchain(b + 1)
                qk_chunk(b, tt, ck, s_prs, st, fine=fine)
                # stage prefetch + next Q-chain into fixed slots
                if tt == 0 and ck == 1 and b + 2 < BS:
                    load_batch(b + 2)
                elif tt == 1 and ck == 0 and b == 0 and b + 1 < BS:
                    qchain(b + 1)
            if fine:
                finish_rowtile_fine(b, tt, s_prs, st)
            else:
                finish_rowtile(b, tt, s_prs, st)
        fst_t.pop(b, None)
        qct_t.pop(b, None)


def build_nc():
    nc = bacc.Bacc(
        "TRN2",
        target_bir_lowering=False,
        debug=False,
        num_devices=N_CORES,
    )
    F_aT = nc.dram_tensor("F_aT", [BS, D, T], BF16, kind="ExternalInput")
    F_sT = nc.dram_tensor("F_sT", [BS, D, HW], BF16, kind="ExternalInput")
    mbig = nc.dram_tensor("mbig", [BS, HW], BF16, kind="ExternalInput")
    Wc = nc.dram_tensor("Wc", [D, D], BF16, kind="ExternalInput")
    bc = nc.dram_tensor("bc", [D], F32, kind="ExternalInput")
    S = nc.dram_tensor("S", [BS, T, HW], BF16, kind="ExternalOutput")

    with tile.TileContext(nc) as tc, ExitStack() as ctx:
        _build_body(
            tc, ctx, F_aT.ap(), F_sT.ap(), mbig.ap(), Wc.ap(), bc.ap(), S.ap()
        )
    nc.compile()
    return nc


def make_in_maps(F_a, F_s, M_s, Wq, bq, Wk):
    F_a = np.asarray(F_a, dtype=np.float32).astype(ml_dtypes.bfloat16)
    F_s = np.asarray(F_s, dtype=np.float32).astype(ml_dtypes.bfloat16)
    M_s = np.asarray(M_s)
    Wqf = np.asarray(Wq, dtype=np.float32)
    Wkf = np.asarray(Wk, dtype=np.float32)
    bqf = np.asarray(bq, dtype=np.float32)
    # Fold: Q~ = F_a @ Wc + bc with scale pre-applied (host-side weights math)
    Wc = np.ascontiguousarray(
        ((Wqf.T @ Wkf) * np.float32(SCALE)).astype(ml_dtypes.bfloat16)
    )
    bc = np.ascontiguousarray(((bqf @ Wkf) * np.float32(SCALE)).astype(np.float32))

    # device-friendly transposed layouts (d on the partition axis)
    F_aT = np.ascontiguousarray(F_a.transpose(0, 2, 1))  # [B, d, T]
    F_sT = np.ascontiguousarray(F_s.transpose(0, 2, 1))  # [B, d, HW]

    m = M_s.reshape(M_s.shape[0], -1) == 1  # [B, HW]
    mbig = np.where(m, np.float32(0.0), np.float32(MASK_NEG)).astype(
        ml_dtypes.bfloat16
    )

    in_maps = []
    for i in range(N_CORES):
        sl = slice(i * BS, (i + 1) * BS)
        in_maps.append(
            dict(
                F_aT=np.ascontiguousarray(F_aT[sl]),
                F_sT=np.ascontiguousarray(F_sT[sl]),
                mbig=np.ascontiguousarray(mbig[sl]),
                Wc=Wc,
                bc=bc,
            )
        )
    return in_maps


_NC_CACHE = None


def _get_nc():
    global _NC_CACHE
    if _NC_CACHE is None:
        _NC_CACHE = build_nc()
    return _NC_CACHE


def run(in_maps, **kwargs):
    from concourse import bass_utils

    nc = _get_nc()
    res = bass_utils.run_bass_kernel_spmd(
        nc, in_maps, core_ids=list(range(N_CORES)), **kwargs
    )
    return res


def kernel(F_a, F_s, M_s, Wq, bq, Wk, bk):
    in_maps = make_in_maps(F_a, F_s, M_s, Wq, bq, Wk)
    res = run(in_maps)
    return np.concatenate(
        [np.asarray(r["S"]).astype(np.float32) for r in res.results], axis=0
    )
